# revision 2
# baseline (speedup 1.0000x reference)
"""Fused LSTM cell on 8 Trainium2 NeuronCores.

Data-parallel over the batch: each core handles 1024 of the 8192 rows.
Per core, the two GEMMs (x @ Wx.T + h @ Wh.T) are fused into one
[2048]-contraction GEMM in float32r (full-rate fp32 PE mode), with the
gate nonlinearities + state update fused into the PSUM eviction path.

Layouts are chosen so every DMA is a plain strided copy (no on-chip
transposes): activations and weights are pre-transposed on the host so
the contraction dim lands on SBUF partitions, and the whole kernel runs
in [hidden, batch] layout; the host transposes the outputs back.
"""

import os

import numpy as np

import concourse.bacc as bacc
import concourse.mybir as mybir
import concourse.tile as tile
from concourse.bass_utils import run_bass_kernel_spmd

B, I, H = 8192, 1024, 1024
NCORES = 8
BL = B // NCORES        # batch rows per core
G4 = 4 * H              # stacked gate dim
KC = (I + H) // 128     # contraction chunks of 128
HT = H // 128           # h-tiles per core
NBC = 2                 # batch chunks per h-tile
BCW = BL // NBC         # 512 columns per matmul (one PSUM bank)

F32 = mybir.dt.float32
F32R = mybir.dt.float32r
AF = mybir.ActivationFunctionType
OP = mybir.AluOpType

_CACHE: dict = {}


def _build():
    nc = bacc.Bacc("TRN2", target_bir_lowering=False, debug=False)
    aT = nc.dram_tensor("a_t", [I + H, BL], F32R, kind="ExternalInput")
    wT = nc.dram_tensor("w_t", [I + H, G4], F32R, kind="ExternalInput")
    cT = nc.dram_tensor("c_t", [H, BL], F32, kind="ExternalInput")
    bias = nc.dram_tensor("bias", [128, 4 * HT], F32, kind="ExternalInput")
    cO = nc.dram_tensor("c_out", [H, BL], F32, kind="ExternalOutput")
    hO = nc.dram_tensor("h_out", [H, BL], F32, kind="ExternalOutput")

    with tile.TileContext(nc) as tc:
        with (
            tc.tile_pool(name="resident", bufs=1) as res_pool,
            tc.tile_pool(name="wpool", bufs=2) as w_pool,
            tc.tile_pool(name="cpool", bufs=2) as c_pool,
            tc.tile_pool(name="opool", bufs=2) as o_pool,
            tc.tile_pool(name="act", bufs=3) as act_pool,
            tc.tile_pool(name="psum", bufs=2, space="PSUM") as psum_pool,
        ):
            # Activations resident for the whole kernel: [128, 16, 1024]
            a_sb = res_pool.tile([128, KC, BL], F32R)
            nc.sync.dma_start(a_sb[:], aT.rearrange("(c p) b -> p c b", p=128))
            bias_sb = res_pool.tile([128, 4 * HT], F32)
            nc.sync.dma_start(bias_sb[:], bias[:])

            # [p, kchunk, gate, htile, col]
            w_r = wT.rearrange("(c p) (G t g) -> p c G t g", p=128, G=4, g=128)

            for t in range(HT):
                w_sb = w_pool.tile([128, KC, 4, 128], F32R, tag="w")
                for g in range(4):
                    nc.sync.dma_start(w_sb[:, :, g, :], w_r[:, :, g, t, :])
                cp_sb = c_pool.tile([128, BL], F32, tag="cprev")
                nc.sync.dma_start(cp_sb[:], cT[t * 128:(t + 1) * 128, :])
                oc_sb = o_pool.tile([128, BL], F32, tag="oc")
                oh_sb = o_pool.tile([128, BL], F32, tag="oh")

                for bc in range(NBC):
                    bsl = slice(bc * BCW, (bc + 1) * BCW)
                    ps = []
                    for g in range(4):
                        p_t = psum_pool.tile([128, BCW], F32, tag=f"ps{g}")
                        for c in range(KC):
                            nc.tensor.matmul(
                                p_t[:], w_sb[:, c, g, :], a_sb[:, c, bsl],
                                start=(c == 0), stop=(c == KC - 1),
                            )
                        ps.append(p_t)

                    si = act_pool.tile([128, BCW], F32, tag="si")
                    sf = act_pool.tile([128, BCW], F32, tag="sf")
                    so = act_pool.tile([128, BCW], F32, tag="so")
                    tg = act_pool.tile([128, BCW], F32, tag="tg")
                    nc.scalar.activation(si[:], ps[0][:], AF.Sigmoid,
                                         bias=bias_sb[:, 0 * HT + t:0 * HT + t + 1])
                    nc.scalar.activation(sf[:], ps[1][:], AF.Sigmoid,
                                         bias=bias_sb[:, 1 * HT + t:1 * HT + t + 1])
                    nc.scalar.activation(so[:], ps[2][:], AF.Sigmoid,
                                         bias=bias_sb[:, 2 * HT + t:2 * HT + t + 1])
                    nc.scalar.activation(tg[:], ps[3][:], AF.Tanh,
                                         bias=bias_sb[:, 3 * HT + t:3 * HT + t + 1])

                    t1 = act_pool.tile([128, BCW], F32, tag="t1")
                    t2 = act_pool.tile([128, BCW], F32, tag="t2")
                    nc.vector.tensor_tensor(t1[:], sf[:], cp_sb[:, bsl], OP.mult)
                    nc.vector.tensor_tensor(t2[:], si[:], tg[:], OP.mult)
                    nc.vector.tensor_tensor(oc_sb[:, bsl], t1[:], t2[:], OP.add)
                    tct = act_pool.tile([128, BCW], F32, tag="tct")
                    nc.scalar.activation(tct[:], oc_sb[:, bsl], AF.Tanh)
                    nc.vector.tensor_tensor(oh_sb[:, bsl], so[:], tct[:], OP.mult)

                nc.sync.dma_start(cO[t * 128:(t + 1) * 128, :], oc_sb[:])
                nc.sync.dma_start(hO[t * 128:(t + 1) * 128, :], oh_sb[:])

    nc.finalize()
    return nc


def kernel(x_current, c_previous, h_previous, Wx, bx, Wh, bh):
    x = np.asarray(x_current, dtype=np.float32)
    c = np.asarray(c_previous, dtype=np.float32)
    h = np.asarray(h_previous, dtype=np.float32)
    Wx = np.asarray(Wx, dtype=np.float32)
    Wh = np.asarray(Wh, dtype=np.float32)
    bsum = np.asarray(bx, dtype=np.float32) + np.asarray(bh, dtype=np.float32)

    wT = np.ascontiguousarray(
        np.concatenate([Wx, Wh], axis=1).T)          # [2048, 4096]
    bias_t = np.ascontiguousarray(bsum.reshape(4 * HT, 128).T)  # [128, 32]

    in_maps = []
    for core in range(NCORES):
        sl = slice(core * BL, (core + 1) * BL)
        aT = np.ascontiguousarray(
            np.concatenate([x[sl], h[sl]], axis=1).T)  # [2048, BL]
        in_maps.append({
            "a_t": aT,
            "w_t": wT,
            "c_t": np.ascontiguousarray(c[sl].T),
            "bias": bias_t,
        })

    if "nc" not in _CACHE:
        _CACHE["nc"] = _build()
    nc = _CACHE["nc"]

    res = run_bass_kernel_spmd(
        nc, in_maps, list(range(NCORES)),
        trace=bool(int(os.environ.get("LSTM_TRACE", "0"))),
    )
    _CACHE["last_result"] = res

    c_out = np.empty((B, H), dtype=np.float32)
    h_out = np.empty((B, H), dtype=np.float32)
    for core in range(NCORES):
        sl = slice(core * BL, (core + 1) * BL)
        c_out[sl] = res.results[core]["c_out"].T
        h_out[sl] = res.results[core]["h_out"].T
    return c_out, h_out


# revision 4
# speedup vs baseline: 1.3013x; 1.3013x over previous
"""Fused LSTM cell on 8 Trainium2 NeuronCores.

Data-parallel over the batch: each core handles 1024 of the 8192 rows.
Per core, the two GEMMs (x @ Wx.T + h @ Wh.T) are fused into one
[2048]-contraction GEMM in float32r (full-rate fp32 PE mode), with the
gate nonlinearities + state update fused into the PSUM eviction path.

Layouts are chosen so every DMA is a plain strided copy (no on-chip
transposes): activations and weights are pre-transposed on the host so
the contraction dim lands on SBUF partitions, and the whole kernel runs
in [hidden, batch] layout; the host transposes the outputs back.
"""

import os

import numpy as np

import concourse.bacc as bacc
import concourse.mybir as mybir
import concourse.tile as tile
from concourse.bass_utils import run_bass_kernel_spmd

B, I, H = 8192, 1024, 1024
NCORES = 8
BL = B // NCORES        # batch rows per core
G4 = 4 * H              # stacked gate dim
KC = (I + H) // 128     # contraction chunks of 128
HT = H // 128           # h-tiles per core
NBC = 2                 # batch chunks per h-tile
BCW = BL // NBC         # 512 columns per matmul (one PSUM bank)

F32 = mybir.dt.float32
F32R = mybir.dt.float32r
AF = mybir.ActivationFunctionType
OP = mybir.AluOpType

_CACHE: dict = {}


def _build(reps=1):
    nc = bacc.Bacc("TRN2", target_bir_lowering=False, debug=False)
    aT = nc.dram_tensor("a_t", [I + H, BL], F32R, kind="ExternalInput")
    wT = nc.dram_tensor("w_t", [I + H, G4], F32R, kind="ExternalInput")
    cT = nc.dram_tensor("c_t", [H, BL], F32, kind="ExternalInput")
    bias = nc.dram_tensor("bias", [128, 4 * HT], F32, kind="ExternalInput")
    cO = nc.dram_tensor("c_out", [H, BL], F32, kind="ExternalOutput")
    hO = nc.dram_tensor("h_out", [H, BL], F32, kind="ExternalOutput")

    with tile.TileContext(nc) as tc:
        with (
            tc.tile_pool(name="resident", bufs=1) as res_pool,
            tc.tile_pool(name="wpool", bufs=2) as w_pool,
            tc.tile_pool(name="cpool", bufs=2) as c_pool,
            tc.tile_pool(name="opool", bufs=2) as o_pool,
            tc.tile_pool(name="act", bufs=3) as act_pool,
            tc.tile_pool(name="psum", bufs=2, space="PSUM") as psum_pool,
        ):
            # Activations resident for the whole kernel: [128, 16, 1024]
            a_sb = res_pool.tile([128, KC, BL], F32R)
            nc.sync.dma_start(a_sb[:], aT.rearrange("(c p) b -> p c b", p=128))
            bias_sb = res_pool.tile([128, 4 * HT], F32)
            nc.sync.dma_start(bias_sb[:], bias[:])

            # [p, kchunk, gate, htile, col]
            w_r = wT.rearrange("(c p) (G t g) -> p c G t g", p=128, G=4, g=128)

            for t in [t for _ in range(reps) for t in range(HT)]:
                w_sb = w_pool.tile([128, KC, 4, 128], F32R, tag="w")
                for g in range(4):
                    nc.sync.dma_start(w_sb[:, :, g, :], w_r[:, :, g, t, :])
                cp_sb = c_pool.tile([128, BL], F32, tag="cprev")
                nc.sync.dma_start(cp_sb[:], cT[t * 128:(t + 1) * 128, :])
                oc_sb = o_pool.tile([128, BL], F32, tag="oc")
                oh_sb = o_pool.tile([128, BL], F32, tag="oh")

                for bc in range(NBC):
                    bsl = slice(bc * BCW, (bc + 1) * BCW)
                    ps = []
                    for g in range(4):
                        p_t = psum_pool.tile([128, BCW], F32, tag=f"ps{g}")
                        for c in range(KC):
                            nc.tensor.matmul(
                                p_t[:], w_sb[:, c, g, :], a_sb[:, c, bsl],
                                start=(c == 0), stop=(c == KC - 1),
                            )
                        ps.append(p_t)

                    si = act_pool.tile([128, BCW], F32, tag="si")
                    sf = act_pool.tile([128, BCW], F32, tag="sf")
                    so = act_pool.tile([128, BCW], F32, tag="so")
                    tg = act_pool.tile([128, BCW], F32, tag="tg")
                    nc.scalar.activation(si[:], ps[0][:], AF.Sigmoid,
                                         bias=bias_sb[:, 0 * HT + t:0 * HT + t + 1])
                    nc.scalar.activation(sf[:], ps[1][:], AF.Sigmoid,
                                         bias=bias_sb[:, 1 * HT + t:1 * HT + t + 1])
                    nc.scalar.activation(so[:], ps[2][:], AF.Sigmoid,
                                         bias=bias_sb[:, 2 * HT + t:2 * HT + t + 1])
                    nc.scalar.activation(tg[:], ps[3][:], AF.Tanh,
                                         bias=bias_sb[:, 3 * HT + t:3 * HT + t + 1])

                    t1 = act_pool.tile([128, BCW], F32, tag="t1")
                    t2 = act_pool.tile([128, BCW], F32, tag="t2")
                    nc.vector.tensor_tensor(t1[:], sf[:], cp_sb[:, bsl], OP.mult)
                    nc.vector.tensor_tensor(t2[:], si[:], tg[:], OP.mult)
                    nc.vector.tensor_tensor(oc_sb[:, bsl], t1[:], t2[:], OP.add)
                    tct = act_pool.tile([128, BCW], F32, tag="tct")
                    nc.scalar.activation(tct[:], oc_sb[:, bsl], AF.Tanh)
                    nc.vector.tensor_tensor(oh_sb[:, bsl], so[:], tct[:], OP.mult)

                nc.sync.dma_start(cO[t * 128:(t + 1) * 128, :], oc_sb[:])
                nc.sync.dma_start(hO[t * 128:(t + 1) * 128, :], oh_sb[:])

    nc.finalize()
    return nc


def kernel(x_current, c_previous, h_previous, Wx, bx, Wh, bh):
    x = np.asarray(x_current, dtype=np.float32)
    c = np.asarray(c_previous, dtype=np.float32)
    h = np.asarray(h_previous, dtype=np.float32)
    Wx = np.asarray(Wx, dtype=np.float32)
    Wh = np.asarray(Wh, dtype=np.float32)
    bsum = np.asarray(bx, dtype=np.float32) + np.asarray(bh, dtype=np.float32)

    wT = np.ascontiguousarray(
        np.concatenate([Wx, Wh], axis=1).T)          # [2048, 4096]
    bias_t = np.ascontiguousarray(bsum.reshape(4 * HT, 128).T)  # [128, 32]

    in_maps = []
    for core in range(NCORES):
        sl = slice(core * BL, (core + 1) * BL)
        aT = np.ascontiguousarray(
            np.concatenate([x[sl], h[sl]], axis=1).T)  # [2048, BL]
        in_maps.append({
            "a_t": aT,
            "w_t": wT,
            "c_t": np.ascontiguousarray(c[sl].T),
            "bias": bias_t,
        })

    if "nc" not in _CACHE:
        _CACHE["nc"] = _build()
    nc = _CACHE["nc"]

    res = run_bass_kernel_spmd(
        nc, in_maps, list(range(NCORES)),
        trace=bool(int(os.environ.get("LSTM_TRACE", "0"))),
    )
    _CACHE["last_result"] = res

    c_out = np.empty((B, H), dtype=np.float32)
    h_out = np.empty((B, H), dtype=np.float32)
    for core in range(NCORES):
        sl = slice(core * BL, (core + 1) * BL)
        c_out[sl] = res.results[core]["c_out"].T
        h_out[sl] = res.results[core]["h_out"].T
    return c_out, h_out
